# revision 7
# baseline (speedup 1.0000x reference)
"""Trainium2 Bass kernel for the GNN bi-interaction aggregator.

side = segment_sum(ego[edge_cols] * edge_vals, edge_rows)
out  = leaky_relu((ego + side) @ W1.T + b1) + leaky_relu((ego * side) @ W2.T + b2)

Design: destination nodes split across 8 cores; the embedding table is
replicated in fp8(e4m3) so each SWDGE gather descriptor moves a 256B row
(the gather is descriptor-rate bound: ~6.5 ns/desc per queue x 4 queues,
so smaller rows and exactly one ~7k-idx call per (group, chunk) minimize
descriptor count and per-call overheads; greedy queue load balancing).

Per core, fully transposed dataflow so everything hides under the gather:
  - Host packs edges into 128-idx gather slots that may cross dest-block
    boundaries; each slot is split into fragments (block, wlo) and every
    fragment gets a host-precomputed narrow S tile [128, W_S] holding
    edge_vals at (edge_row, dest-wlo).  sideT[d, node] accumulates in PSUM
    via matmul(lhsT=G[:, slot, half], rhs=S) at free-dim offset wlo --
    ~16-24 cycles per fragment instead of 256.
  - MLP transposed: zT = W @ xT as 8 bf16 matmuls per block (no PE
    transposes, no PSUM->SBUF copy); bias seeded by K=1 matmuls; leaky
    relu on the Activation engine (Lrelu, alpha=0.01); DVE does only the
    two elementwise inputs and the final add.
  - egoT / outT in bf16 block layout [nb, 128(d), 2(half), 128(node)];
    the host packs/unpacks (free).
"""
import sys
import threading

import numpy as np

if "/opt/trn_rl_repo" not in sys.path:
    sys.path.append("/opt/trn_rl_repo")

import ml_dtypes  # noqa: E402
import concourse.bass as bass  # noqa: E402
import concourse.bacc as bacc  # noqa: E402
import concourse.mybir as mybir  # noqa: E402
from concourse.tile import TileContext  # noqa: E402

P = 128
D = 256
W_S = 24            # dest-window width of a narrow S tile
N_CORES = 8
CHUNK = 25000       # table chunk rows (int16 gather index limit)
MAX_CALL_TILES = 64  # <=8192 idxs per dma_gather call
GROUP_BLOCKS = 7    # dest blocks per gather group
NQUEUES = 4
SINGLE_PACKET = False
TABLE_DT = 'fp8'    # 'bf16' | 'fp8'
GBUFS = 2
F32 = mybir.dt.float32
BF16 = mybir.dt.bfloat16
F8 = mybir.dt.float8e4
I16 = mybir.dt.int16
AL = mybir.AluOpType
AF = mybir.ActivationFunctionType
NEG_SLOPE = 0.01
LRELU_MODE = 'act_lrelu'   # 'act_lrelu' | 'act_relu2'
_LAST_RUNNERS = []
_LAST_NCS = []


# ---------------- host preprocessing ----------------

def preprocess_core(rows, cols, vals, lo, hi):
    """Static tile/gather structure for destination rows [lo, hi).

    Slots of 128 gather idxs packed across block boundaries; each slot is
    split into fragments (block, wlo, S col) -> one narrow matmul each.
    """
    rows = np.asarray(rows); cols = np.asarray(cols); vals = np.asarray(vals)
    sel = (rows >= lo) & (rows < hi)
    r, c, v = rows[sel] - lo, cols[sel], vals[sel]
    order = np.argsort(r, kind='stable')
    r, c, v = r[order], c[order], v[order]
    nnodes = hi - lo
    nblocks = (nnodes + P - 1) // P
    nchunks = 4

    idx16_cols = []
    S_cols = []
    groups = []
    gslot = 0
    gmm = 0
    for g0 in range(0, nblocks, GROUP_BLOCKS):
        g1 = min(g0 + GROUP_BLOCKS, nblocks)
        in_grp = (r >= g0 * P) & (r < g1 * P)
        rg, cg, vg = r[in_grp], c[in_grp], v[in_grp]
        # phase chunking (col % 4): each chunk's rows interleave across the
        # full table span, so a call's 128-desc in-flight window spreads over
        # 4x the address range (measured ~10% faster than range chunks)
        ch = cg % 4
        slots_idx = []        # per slot: idx128 (chunk-relative), chunk
        mms = []              # (slot_local, block, wlo, S128xW)
        calls = []            # (chunk, s0, ntiles, icol0_global)
        for cc in range(nchunks):
            m = ch == cc
            if not m.any():
                continue
            rc, cx, vx = rg[m], cg[m] // 4, vg[m]
            o2 = np.argsort(rc, kind='stable')
            rc, cx, vx = rc[o2], cx[o2], vx[o2]
            n = len(rc)
            pos = 0
            chunk_s0 = len(slots_idx)
            while pos < n:
                # build one slot (<=128 edges, fragments)
                cap = min(P, n - pos)
                idx128 = np.empty(P, np.int64)
                fill = 0
                while fill < cap:
                    b = int(rc[pos + fill]) // P
                    brel = rc[pos + fill] - b * P
                    wlo = min(int(brel), P - W_S)
                    # edges of block b with dest < wlo+W_S, within slot space
                    lim_dest = b * P + wlo + W_S
                    stop = int(np.searchsorted(rc, lim_dest, side='left'))
                    stop = min(stop, pos + cap)
                    take = stop - (pos + fill)
                    sl = slice(pos + fill, pos + fill + take)
                    S = np.zeros((P, W_S), np.float32)
                    S[np.arange(fill, fill + take),
                      rc[sl] - b * P - wlo] = vx[sl]
                    mms.append((len(slots_idx), b, wlo, S))
                    idx128[fill:fill + take] = cx[sl]
                    fill += take
                idx128[fill:] = idx128[0]
                slots_idx.append((idx128, cc))
                pos += cap
            # calls for this chunk run
            s0 = chunk_s0
            while s0 < len(slots_idx):
                ntiles = min(MAX_CALL_TILES, len(slots_idx) - s0)
                flat = np.concatenate(
                    [slots_idx[s][0] for s in range(s0, s0 + ntiles)])
                assert flat.min() >= 0 and flat.max() < CHUNK
                i16 = np.tile(flat.reshape(-1, 16).T.astype(np.int16), (8, 1))
                calls.append((cc, s0, ntiles,
                              sum(x.shape[1] for x in idx16_cols)))
                idx16_cols.append(i16)
                s0 += ntiles
        # per-block mm lists (mm_local, slot_local, wlo)
        binfo = []
        for b in range(g0, g1):
            bmms = [(mi, mm[0], mm[2]) for mi, mm in enumerate(mms)
                    if mm[1] == b]
            binfo.append((b, bmms))
        for mm in mms:
            S_cols.append(mm[3])
        groups.append(dict(calls=calls, slot0=gslot, nslots=len(slots_idx),
                           mm0=gmm, nmms=len(mms), blocks=binfo))
        gslot += len(slots_idx)
        gmm += len(mms)

    sdt = ml_dtypes.float8_e4m3 if (TABLE_DT == 'fp8') else ml_dtypes.bfloat16
    idx16 = (np.concatenate(idx16_cols, axis=1) if idx16_cols
             else np.zeros((P, 16), np.int16))
    Sflat = (np.stack(S_cols, axis=1).astype(sdt) if S_cols
             else np.zeros((P, 1, W_S), sdt))
    return dict(idx16=idx16, Sflat=Sflat, groups=groups,
                nblocks=nblocks, nnodes=nnodes, nslots=gslot, nmms=gmm)


def make_table(ego):
    ego = np.asarray(ego, np.float32)
    if TABLE_DT == 'fp8':
        # rows at non-power-of-2 768B pitch, grouped [n//4, 4, 768] for
        # phase chunks (col % 4): both de-correlate HBM banks inside the
        # gather's 128-desc in-flight window
        nr = (ego.shape[0] + 3) // 4 * 4
        t = np.zeros((nr, 3 * D), ml_dtypes.float8_e4m3)
        t[:ego.shape[0], :D] = ego.astype(ml_dtypes.float8_e4m3)
        return t.reshape(nr // 4, 4, 3 * D)
    return ego.astype(ml_dtypes.bfloat16)


def make_core_inputs(struct, table_bf16, ego_slice, W1, b1, W2, b2):
    nb = struct["nblocks"]
    nnodes = struct["nnodes"]
    npad = nb * P
    egoT = np.zeros((npad, D), np.float32)
    egoT[:nnodes] = ego_slice
    # [nb, 128(node), 2(half), 128(dpart)] -> [nb, 128(dpart), 2, 128(node)]
    egoT = egoT.reshape(nb, P, 2, P).transpose(0, 3, 2, 1)
    egoT = np.ascontiguousarray(egoT).astype(ml_dtypes.bfloat16)
    # w12: lhsT slices [dk(128 part), k-half, (branch,jhalf) 4*128]
    w12 = np.zeros((P, 2, 4 * P), np.float32)
    for br, W in enumerate((W1, W2)):
        WT = W.astype(np.float32).T          # [d, j]
        for k in range(2):
            w12[:, k, br * D:(br + 1) * D] = WT[k * P:(k + 1) * P, :]
    bvec = np.concatenate([b1, b2]).astype(np.float32).reshape(1, 4 * P)
    return {
        "table": table_bf16,
        "egoT": egoT,
        "idx16": struct["idx16"],
        "Sflat": np.ascontiguousarray(struct["Sflat"]),
        "w12": w12.astype(ml_dtypes.bfloat16),
        "bvec": bvec.astype(ml_dtypes.bfloat16),
        "ones": np.ones((1, P), ml_dtypes.bfloat16),
        "zeros": np.zeros((1, D), ml_dtypes.bfloat16),
    }


# ---------------- program builder ----------------

def build_core_program(struct, n_table_rows, gbufs=None, reps=1, stage='full'):
    if gbufs is None:
        gbufs = GBUFS
    nb = struct["nblocks"]
    Ti = struct["idx16"].shape[1]
    St = struct["Sflat"].shape[1]
    groups = struct["groups"]

    GDT = F8 if TABLE_DT == 'fp8' else BF16
    trow = 3 * D if TABLE_DT == 'fp8' else D
    nrows4 = (n_table_rows + 3) // 4
    nc = bacc.Bacc("TRN2", target_bir_lowering=False, debug=False,
                   num_swdge_queues=NQUEUES)
    table = nc.dram_tensor("table", [nrows4, 4, trow], GDT,
                           kind="ExternalInput")
    egoT = nc.dram_tensor("egoT", [nb, P, 2, P], BF16, kind="ExternalInput")
    idx16 = nc.dram_tensor("idx16", [P, Ti], I16, kind="ExternalInput")
    Sflat = nc.dram_tensor("Sflat", [P, St, W_S], GDT, kind="ExternalInput")
    w12 = nc.dram_tensor("w12", [P, 2, 4 * P], BF16, kind="ExternalInput")
    bvec = nc.dram_tensor("bvec", [1, 4 * P], BF16, kind="ExternalInput")
    ones = nc.dram_tensor("ones", [1, P], BF16, kind="ExternalInput")
    zeros = nc.dram_tensor("zeros", [1, D], BF16, kind="ExternalInput")
    outT = nc.dram_tensor("outT", [nb, P, 2, P], BF16, kind="ExternalOutput")

    with TileContext(nc) as tc:
        with (
            tc.tile_pool(name="const", bufs=1) as cpool,
            tc.tile_pool(name="g", bufs=gbufs) as gpool,
            tc.tile_pool(name="s", bufs=gbufs) as spool,
            tc.tile_pool(name="e", bufs=3) as epool,
            tc.tile_pool(name="m", bufs=3) as mpool,
            tc.tile_pool(name="pside", bufs=2, space="PSUM") as pside_pool,
            tc.tile_pool(name="pz", bufs=2, space="PSUM") as pz_pool,
        ):
            w12_sb = cpool.tile([P, 2, 4 * P], BF16)
            nc.sync.dma_start(out=w12_sb[:], in_=w12[:, :, :])
            bvec_sb = cpool.tile([1, 4 * P], BF16)
            nc.sync.dma_start(out=bvec_sb[:], in_=bvec[:, :])
            ones_sb = cpool.tile([1, P], BF16)
            nc.sync.dma_start(out=ones_sb[:], in_=ones[:, :])
            zeros_sb = cpool.tile([1, D], BF16)
            nc.sync.dma_start(out=zeros_sb[:], in_=zeros[:, :])

            qload = [0] * NQUEUES
            for _rep in range(reps):
              for g in groups:
                nslots = g["nslots"]
                slot0 = g["slot0"]
                nmms = g["nmms"]
                mm0 = g["mm0"]
                if nslots:
                    g_icol0 = g["calls"][0][3]
                    g_icols = nslots * P // 16
                    idx_sb = spool.tile([P, g_icols], I16, tag="idx")
                    nc.sync.dma_start(
                        out=idx_sb[:], in_=idx16[:, g_icol0:g_icol0 + g_icols])
                    G = gpool.tile([P, nslots, D], GDT, tag="G")
                    for (cc, s0, ntiles, icol0) in g["calls"]:
                        nidx = ntiles * P
                        li = icol0 - g_icol0
                        qsel = min(range(NQUEUES), key=lambda q: qload[q])
                        nc.gpsimd.dma_gather(
                            out_ap=G[:, s0:s0 + ntiles, :],
                            in_ap=table[:, cc, :D],
                            idxs_ap=idx_sb[:, li:li + nidx // 16],
                            num_idxs=nidx, num_idxs_reg=nidx, elem_size=D,
                            elem_step=4 * trow,
                            single_packet=SINGLE_PACKET, queue_num=qsel)
                        qload[qsel] += nidx
                    if stage == 'gather':
                        continue
                    S_sb = spool.tile([P, nmms, W_S], GDT, tag="S")
                    nc.sync.dma_start(out=S_sb[:],
                                      in_=Sflat[:, mm0:mm0 + nmms, :])
                elif stage == 'gather':
                    continue
                # ---- per-block SpMM + MLP
                for (b, bmms) in g["blocks"]:
                    eg = epool.tile([P, 2, P], BF16, tag="eg")
                    nc.sync.dma_start(out=eg[:], in_=egoT[b, :, :, :])
                    pside = pside_pool.tile([P, 4, P], F32, tag="side")
                    nc.tensor.matmul(out=pside[:, :, :],
                                     lhsT=zeros_sb[:, :P],
                                     rhs=zeros_sb[:, :].unsqueeze(1)
                                         .broadcast_to([1, 2, D]),
                                     start=True, stop=(not bmms),
                                     skip_group_check=True)
                    for i, (mi, si, wlo) in enumerate(bmms):
                        last = i == len(bmms) - 1
                        for h in range(2):
                            nc.tensor.matmul(
                                out=pside[:, h, wlo:wlo + W_S],
                                lhsT=G[:, si, h * P:(h + 1) * P],
                                rhs=S_sb[:, mi, :],
                                start=False, stop=(last and h == 1),
                                skip_group_check=True)
                    sum_in = mpool.tile([P, 2, P], BF16, tag="sum_in")
                    nc.vector.tensor_tensor(out=sum_in[:], in0=pside[:, 0:2, :],
                                            in1=eg[:], op=AL.add)
                    bi_in = mpool.tile([P, 2, P], BF16, tag="bi_in")
                    nc.vector.tensor_tensor(out=bi_in[:], in0=pside[:, 0:2, :],
                                            in1=eg[:], op=AL.mult)
                    pz = pz_pool.tile([P, 4, P], F32, tag="Z")
                    for i in range(4):
                        nc.tensor.matmul(out=pz[:, i, :],
                                         lhsT=bvec_sb[:, i * P:(i + 1) * P],
                                         rhs=ones_sb[:, :],
                                         start=(i == 0), stop=False,
                                         skip_group_check=True)
                    for br, xin in ((0, sum_in), (1, bi_in)):
                        for k in range(2):
                            for jh in range(2):
                                nc.tensor.matmul(
                                    out=pz[:, br * 2 + jh, :],
                                    lhsT=w12_sb[:, k,
                                                br * D + jh * P:
                                                br * D + (jh + 1) * P],
                                    rhs=xin[:, k, :],
                                    start=False, stop=(k == 1),
                                    skip_group_check=True)
                    ob = mpool.tile([P, 2, P], BF16, tag="ob")
                    if LRELU_MODE == 'act_lrelu':
                        ot = mpool.tile([P, 4, P], BF16, tag="ot")
                        nc.scalar.activation(out=ot[:, 0:2, :], in_=pz[:, 0:2, :],
                                             func=AF.Lrelu, alpha=NEG_SLOPE)
                        nc.scalar.activation(out=ot[:, 2:4, :], in_=pz[:, 2:4, :],
                                             func=AF.Lrelu, alpha=NEG_SLOPE)
                        nc.vector.tensor_tensor(out=ob[:], in0=ot[:, 0:2, :],
                                                in1=ot[:, 2:4, :], op=AL.add)
                    else:
                        # LR(z) = Relu(z) - slope*Relu(-z), per branch
                        rp = mpool.tile([P, 4, P], BF16, tag="rp")
                        rn = mpool.tile([P, 4, P], F32, tag="rn")
                        nc.scalar.activation(out=rp[:, 0:2, :], in_=pz[:, 0:2, :],
                                             func=AF.Relu)
                        nc.scalar.activation(out=rp[:, 2:4, :], in_=pz[:, 2:4, :],
                                             func=AF.Relu)
                        nc.scalar.activation(out=rn[:, 0:2, :], in_=pz[:, 0:2, :],
                                             func=AF.Relu, scale=-1.0)
                        nc.scalar.activation(out=rn[:, 2:4, :], in_=pz[:, 2:4, :],
                                             func=AF.Relu, scale=-1.0)
                        nc.vector.tensor_tensor(out=ob[:], in0=rp[:, 0:2, :],
                                                in1=rp[:, 2:4, :], op=AL.add)
                        rs = mpool.tile([P, 2, P], F32, tag="rs")
                        nc.vector.tensor_tensor(out=rs[:], in0=rn[:, 0:2, :],
                                                in1=rn[:, 2:4, :], op=AL.add)
                        nc.vector.tensor_scalar(out=rs[:], in0=rs[:],
                                                scalar1=-NEG_SLOPE, scalar2=None,
                                                op0=AL.mult)
                        nc.vector.tensor_tensor(out=ob[:], in0=ob[:],
                                                in1=rs[:], op=AL.add)
                    nc.sync.dma_start(out=outT[b, :, :, :], in_=ob[:])
    nc.compile()
    return nc


# ---------------- PJRT execution ----------------

def _make_exec(nc, device):
    import jax
    from concourse.bass2jax import _bass_exec_p, install_neuronx_cc_hook
    install_neuronx_cc_hook()
    in_names, out_names, out_avals, zero_outs = [], [], [], []
    in_specs = {}
    for alloc in nc.m.functions[0].allocations:
        if not isinstance(alloc, mybir.MemoryLocationSet):
            continue
        name = alloc.memorylocations[0].name
        if alloc.kind == "ExternalInput":
            in_names.append(name)
            in_specs[name] = (tuple(alloc.tensor_shape), mybir.dt.np(alloc.dtype))
        elif alloc.kind == "ExternalOutput":
            out_names.append(name)
            shape = tuple(alloc.tensor_shape)
            dtype = mybir.dt.np(alloc.dtype)
            out_avals.append(jax.core.ShapedArray(shape, dtype))
            zero_outs.append(np.zeros(shape, dtype))
    all_in_names = in_names + out_names

    def _body(*args):
        outs = _bass_exec_p.bind(
            *args,
            out_avals=tuple(out_avals),
            in_names=tuple(all_in_names),
            out_names=tuple(out_names),
            lowering_input_output_aliases=(),
            sim_require_finite=True,
            sim_require_nnan=True,
            nc=nc,
        )
        return tuple(outs)

    jitted = jax.jit(_body, keep_unused=True, device=device)
    return jitted, in_names, out_names, zero_outs, in_specs


class CoreRunner:
    def __init__(self, nc, device, in_map):
        import jax
        self.jax = jax
        (self.jitted, self.in_names, self.out_names, self.zero_outs,
         in_specs) = _make_exec(nc, device)
        self.dev_in = [
            jax.device_put(
                np.asarray(in_map[n]) if n in in_map
                else np.zeros(*in_specs[n][:1], in_specs[n][1]), device)
            for n in self.in_names]
        self.dev_zero = [jax.device_put(z, device) for z in self.zero_outs]

    def run_async(self):
        return self.jitted(*self.dev_in, *self.dev_zero)

    def outputs_np(self):
        outs = self.jax.block_until_ready(self.run_async())
        return {n: np.asarray(o) for n, o in zip(self.out_names, outs)}


# ---------------- top-level entry ----------------

def kernel(ego_embeddings, edge_vals, W1, b1, W2, b2, edge_rows, edge_cols):
    import jax
    ego = np.asarray(ego_embeddings, np.float32)
    edge_vals = np.asarray(edge_vals, np.float32)
    W1 = np.asarray(W1, np.float32); b1 = np.asarray(b1, np.float32)
    W2 = np.asarray(W2, np.float32); b2 = np.asarray(b2, np.float32)
    rows = np.asarray(edge_rows); cols = np.asarray(edge_cols)
    n = ego.shape[0]
    table_bf16 = make_table(ego)

    bounds = [round(n * c / N_CORES) for c in range(N_CORES + 1)]
    structs = [preprocess_core(rows, cols, edge_vals, bounds[c], bounds[c + 1])
               for c in range(N_CORES)]
    devices = jax.devices()[:N_CORES]

    ncs = [None] * N_CORES
    errs = [None] * N_CORES

    def _build(c):
        try:
            ncs[c] = build_core_program(structs[c], n)
        except Exception as e:  # noqa: BLE001
            errs[c] = e

    threads = [threading.Thread(target=_build, args=(c,)) for c in range(N_CORES)]
    for t in threads:
        t.start()
    for t in threads:
        t.join()
    for e in errs:
        if e is not None:
            raise e

    runners = []
    for c in range(N_CORES):
        in_map = make_core_inputs(structs[c], table_bf16,
                                  ego[bounds[c]:bounds[c + 1]], W1, b1, W2, b2)
        runners.append(CoreRunner(ncs[c], devices[c], in_map))

    global _LAST_RUNNERS, _LAST_NCS
    _LAST_RUNNERS = runners
    _LAST_NCS = ncs
    futs = [r.run_async() for r in runners]
    out = np.empty((n, D), np.float32)
    for c, (r, f) in enumerate(zip(runners, futs)):
        outs = jax.block_until_ready(f)
        oT = np.asarray(outs[r.out_names.index("outT")])  # [nb,128,2,128] bf16
        nnodes = bounds[c + 1] - bounds[c]
        nbb = oT.shape[0]
        full = oT.astype(np.float32).transpose(0, 3, 2, 1).reshape(nbb * P, D)
        out[bounds[c]:bounds[c + 1]] = full[:nnodes]
    return out


# revision 8
# speedup vs baseline: 1.0631x; 1.0631x over previous
"""Trainium2 Bass kernel for the GNN bi-interaction aggregator.

side = segment_sum(ego[edge_cols] * edge_vals, edge_rows)
out  = leaky_relu((ego + side) @ W1.T + b1) + leaky_relu((ego * side) @ W2.T + b2)

Design: destination nodes split across 8 cores; the embedding table is
replicated in fp8(e4m3) so each SWDGE gather descriptor moves a 256B row
(the gather is descriptor-rate bound: ~6.5 ns/desc per queue x 4 queues,
so smaller rows and exactly one ~7k-idx call per (group, chunk) minimize
descriptor count and per-call overheads; greedy queue load balancing).

Per core, fully transposed dataflow so everything hides under the gather:
  - Host packs edges into 128-idx gather slots that may cross dest-block
    boundaries; each slot is split into fragments (block, wlo) and every
    fragment gets a host-precomputed narrow S tile [128, W_S] holding
    edge_vals at (edge_row, dest-wlo).  sideT[d, node] accumulates in PSUM
    via matmul(lhsT=G[:, slot, half], rhs=S) at free-dim offset wlo --
    ~16-24 cycles per fragment instead of 256.
  - MLP transposed: zT = W @ xT as 8 bf16 matmuls per block (no PE
    transposes, no PSUM->SBUF copy); bias seeded by K=1 matmuls; leaky
    relu on the Activation engine (Lrelu, alpha=0.01); DVE does only the
    two elementwise inputs and the final add.
  - egoT / outT in bf16 block layout [nb, 128(d), 2(half), 128(node)];
    the host packs/unpacks (free).
"""
import sys
import threading

import numpy as np

if "/opt/trn_rl_repo" not in sys.path:
    sys.path.append("/opt/trn_rl_repo")

import ml_dtypes  # noqa: E402
import concourse.bass as bass  # noqa: E402
import concourse.bacc as bacc  # noqa: E402
import concourse.mybir as mybir  # noqa: E402
from concourse.tile import TileContext  # noqa: E402

P = 128
D = 256
W_S = 24            # dest-window width of a narrow S tile
N_CORES = 8
CHUNK = 25000       # table chunk rows (int16 gather index limit)
MAX_CALL_TILES = 64  # <=8192 idxs per dma_gather call
GROUP_BLOCKS = 7    # dest blocks per gather group
NQUEUES = 4
SINGLE_PACKET = False
TABLE_DT = 'fp8'    # 'bf16' | 'fp8'
GBUFS = 2
F32 = mybir.dt.float32
BF16 = mybir.dt.bfloat16
F8 = mybir.dt.float8e4
I16 = mybir.dt.int16
AL = mybir.AluOpType
AF = mybir.ActivationFunctionType
NEG_SLOPE = 0.01
LRELU_MODE = 'act_lrelu'   # 'act_lrelu' | 'act_relu2'
_LAST_RUNNERS = []
_LAST_NCS = []


# ---------------- host preprocessing ----------------

def preprocess_core(rows, cols, vals, lo, hi):
    """Static tile/gather structure for destination rows [lo, hi).

    Slots of 128 gather idxs packed across block boundaries; each slot is
    split into fragments (block, wlo, S col) -> one narrow matmul each.
    """
    rows = np.asarray(rows); cols = np.asarray(cols); vals = np.asarray(vals)
    sel = (rows >= lo) & (rows < hi)
    r, c, v = rows[sel] - lo, cols[sel], vals[sel]
    order = np.argsort(r, kind='stable')
    r, c, v = r[order], c[order], v[order]
    nnodes = hi - lo
    nblocks = (nnodes + P - 1) // P
    nchunks = 4

    idx16_cols = []
    S_cols = []
    groups = []
    gslot = 0
    gmm = 0
    for g0 in range(0, nblocks, GROUP_BLOCKS):
        g1 = min(g0 + GROUP_BLOCKS, nblocks)
        in_grp = (r >= g0 * P) & (r < g1 * P)
        rg, cg, vg = r[in_grp], c[in_grp], v[in_grp]
        ch = cg // CHUNK
        slots_idx = []        # per slot: idx128 (chunk-relative), chunk
        mms = []              # (slot_local, block, wlo, S128xW)
        calls = []            # (chunk, s0, ntiles, icol0_global)
        for cc in range(nchunks):
            m = ch == cc
            if not m.any():
                continue
            rc, cx, vx = rg[m], cg[m] - cc * CHUNK, vg[m]
            o2 = np.argsort(rc, kind='stable')
            rc, cx, vx = rc[o2], cx[o2], vx[o2]
            n = len(rc)
            pos = 0
            chunk_s0 = len(slots_idx)
            while pos < n:
                # build one slot (<=128 edges, fragments)
                cap = min(P, n - pos)
                idx128 = np.empty(P, np.int64)
                fill = 0
                while fill < cap:
                    b = int(rc[pos + fill]) // P
                    brel = rc[pos + fill] - b * P
                    wlo = min(int(brel), P - W_S)
                    # edges of block b with dest < wlo+W_S, within slot space
                    lim_dest = b * P + wlo + W_S
                    stop = int(np.searchsorted(rc, lim_dest, side='left'))
                    stop = min(stop, pos + cap)
                    take = stop - (pos + fill)
                    sl = slice(pos + fill, pos + fill + take)
                    S = np.zeros((P, W_S), np.float32)
                    S[np.arange(fill, fill + take),
                      rc[sl] - b * P - wlo] = vx[sl]
                    mms.append((len(slots_idx), b, wlo, S))
                    idx128[fill:fill + take] = cx[sl]
                    fill += take
                idx128[fill:] = idx128[0]
                slots_idx.append((idx128, cc))
                pos += cap
            # calls for this chunk run
            s0 = chunk_s0
            while s0 < len(slots_idx):
                ntiles = min(MAX_CALL_TILES, len(slots_idx) - s0)
                flat = np.concatenate(
                    [slots_idx[s][0] for s in range(s0, s0 + ntiles)])
                assert flat.min() >= 0 and flat.max() < CHUNK
                i16 = np.tile(flat.reshape(-1, 16).T.astype(np.int16), (8, 1))
                calls.append((cc, s0, ntiles,
                              sum(x.shape[1] for x in idx16_cols)))
                idx16_cols.append(i16)
                s0 += ntiles
        # per-block mm lists (mm_local, slot_local, wlo)
        binfo = []
        for b in range(g0, g1):
            bmms = [(mi, mm[0], mm[2]) for mi, mm in enumerate(mms)
                    if mm[1] == b]
            binfo.append((b, bmms))
        for mm in mms:
            S_cols.append(mm[3])
        groups.append(dict(calls=calls, slot0=gslot, nslots=len(slots_idx),
                           mm0=gmm, nmms=len(mms), blocks=binfo))
        gslot += len(slots_idx)
        gmm += len(mms)

    sdt = ml_dtypes.float8_e4m3 if (TABLE_DT == 'fp8') else ml_dtypes.bfloat16
    idx16 = (np.concatenate(idx16_cols, axis=1) if idx16_cols
             else np.zeros((P, 16), np.int16))
    Sflat = (np.stack(S_cols, axis=1).astype(sdt) if S_cols
             else np.zeros((P, 1, W_S), sdt))
    return dict(idx16=idx16, Sflat=Sflat, groups=groups,
                nblocks=nblocks, nnodes=nnodes, nslots=gslot, nmms=gmm)


def make_table(ego):
    ego = np.asarray(ego, np.float32)
    if TABLE_DT == 'fp8':
        # rows at non-power-of-2 768B pitch: de-correlates the row->HBM-bank
        # mapping inside the gather's 128-desc in-flight window
        t = np.zeros((ego.shape[0], 3 * D), ml_dtypes.float8_e4m3)
        t[:, :D] = ego.astype(ml_dtypes.float8_e4m3)
        return t
    return ego.astype(ml_dtypes.bfloat16)


def make_core_inputs(struct, table_bf16, ego_slice, W1, b1, W2, b2):
    nb = struct["nblocks"]
    nnodes = struct["nnodes"]
    npad = nb * P
    egoT = np.zeros((npad, D), np.float32)
    egoT[:nnodes] = ego_slice
    # [nb, 128(node), 2(half), 128(dpart)] -> [nb, 128(dpart), 2, 128(node)]
    egoT = egoT.reshape(nb, P, 2, P).transpose(0, 3, 2, 1)
    egoT = np.ascontiguousarray(egoT).astype(ml_dtypes.bfloat16)
    # w12: lhsT slices [dk(128 part), k-half, (branch,jhalf) 4*128]
    w12 = np.zeros((P, 2, 4 * P), np.float32)
    for br, W in enumerate((W1, W2)):
        WT = W.astype(np.float32).T          # [d, j]
        for k in range(2):
            w12[:, k, br * D:(br + 1) * D] = WT[k * P:(k + 1) * P, :]
    bvec = np.concatenate([b1, b2]).astype(np.float32).reshape(1, 4 * P)
    return {
        "table": table_bf16,
        "egoT": egoT,
        "idx16": struct["idx16"],
        "Sflat": np.ascontiguousarray(struct["Sflat"]),
        "w12": w12.astype(ml_dtypes.bfloat16),
        "bvec": bvec.astype(ml_dtypes.bfloat16),
        "ones": np.ones((1, P), ml_dtypes.bfloat16),
        "zeros": np.zeros((1, D), ml_dtypes.bfloat16),
    }


# ---------------- program builder ----------------

def build_core_program(struct, n_table_rows, gbufs=None, reps=1, stage='full'):
    if gbufs is None:
        gbufs = GBUFS
    nb = struct["nblocks"]
    Ti = struct["idx16"].shape[1]
    St = struct["Sflat"].shape[1]
    groups = struct["groups"]

    GDT = F8 if TABLE_DT == 'fp8' else BF16
    trow = 3 * D if TABLE_DT == 'fp8' else D
    nc = bacc.Bacc("TRN2", target_bir_lowering=False, debug=False,
                   num_swdge_queues=NQUEUES)
    table = nc.dram_tensor("table", [n_table_rows, trow], GDT,
                           kind="ExternalInput")
    egoT = nc.dram_tensor("egoT", [nb, P, 2, P], BF16, kind="ExternalInput")
    idx16 = nc.dram_tensor("idx16", [P, Ti], I16, kind="ExternalInput")
    Sflat = nc.dram_tensor("Sflat", [P, St, W_S], GDT, kind="ExternalInput")
    w12 = nc.dram_tensor("w12", [P, 2, 4 * P], BF16, kind="ExternalInput")
    bvec = nc.dram_tensor("bvec", [1, 4 * P], BF16, kind="ExternalInput")
    ones = nc.dram_tensor("ones", [1, P], BF16, kind="ExternalInput")
    zeros = nc.dram_tensor("zeros", [1, D], BF16, kind="ExternalInput")
    outT = nc.dram_tensor("outT", [nb, P, 2, P], BF16, kind="ExternalOutput")

    with TileContext(nc) as tc:
        with (
            tc.tile_pool(name="const", bufs=1) as cpool,
            tc.tile_pool(name="g", bufs=gbufs) as gpool,
            tc.tile_pool(name="s", bufs=gbufs) as spool,
            tc.tile_pool(name="e", bufs=3) as epool,
            tc.tile_pool(name="m", bufs=3) as mpool,
            tc.tile_pool(name="pside", bufs=2, space="PSUM") as pside_pool,
            tc.tile_pool(name="pz", bufs=2, space="PSUM") as pz_pool,
        ):
            w12_sb = cpool.tile([P, 2, 4 * P], BF16)
            nc.sync.dma_start(out=w12_sb[:], in_=w12[:, :, :])
            bvec_sb = cpool.tile([1, 4 * P], BF16)
            nc.sync.dma_start(out=bvec_sb[:], in_=bvec[:, :])
            ones_sb = cpool.tile([1, P], BF16)
            nc.sync.dma_start(out=ones_sb[:], in_=ones[:, :])
            zeros_sb = cpool.tile([1, D], BF16)
            nc.sync.dma_start(out=zeros_sb[:], in_=zeros[:, :])

            qload = [0] * NQUEUES
            for _rep in range(reps):
              for g in groups:
                nslots = g["nslots"]
                slot0 = g["slot0"]
                nmms = g["nmms"]
                mm0 = g["mm0"]
                if nslots:
                    g_icol0 = g["calls"][0][3]
                    g_icols = nslots * P // 16
                    idx_sb = spool.tile([P, g_icols], I16, tag="idx")
                    nc.sync.dma_start(
                        out=idx_sb[:], in_=idx16[:, g_icol0:g_icol0 + g_icols])
                    G = gpool.tile([P, nslots, D], GDT, tag="G")
                    for (cc, s0, ntiles, icol0) in g["calls"]:
                        nidx = ntiles * P
                        li = icol0 - g_icol0
                        qsel = min(range(NQUEUES), key=lambda q: qload[q])
                        nc.gpsimd.dma_gather(
                            out_ap=G[:, s0:s0 + ntiles, :],
                            in_ap=table[cc * CHUNK:min((cc + 1) * CHUNK,
                                                       n_table_rows), :D],
                            idxs_ap=idx_sb[:, li:li + nidx // 16],
                            num_idxs=nidx, num_idxs_reg=nidx, elem_size=D,
                            elem_step=trow,
                            single_packet=SINGLE_PACKET, queue_num=qsel)
                        qload[qsel] += nidx
                    if stage == 'gather':
                        continue
                    S_sb = spool.tile([P, nmms, W_S], GDT, tag="S")
                    nc.sync.dma_start(out=S_sb[:],
                                      in_=Sflat[:, mm0:mm0 + nmms, :])
                elif stage == 'gather':
                    continue
                # ---- per-block SpMM + MLP
                for (b, bmms) in g["blocks"]:
                    eg = epool.tile([P, 2, P], BF16, tag="eg")
                    nc.sync.dma_start(out=eg[:], in_=egoT[b, :, :, :])
                    pside = pside_pool.tile([P, 4, P], F32, tag="side")
                    nc.tensor.matmul(out=pside[:, :, :],
                                     lhsT=zeros_sb[:, :P],
                                     rhs=zeros_sb[:, :].unsqueeze(1)
                                         .broadcast_to([1, 2, D]),
                                     start=True, stop=(not bmms),
                                     skip_group_check=True)
                    for i, (mi, si, wlo) in enumerate(bmms):
                        last = i == len(bmms) - 1
                        for h in range(2):
                            nc.tensor.matmul(
                                out=pside[:, h, wlo:wlo + W_S],
                                lhsT=G[:, si, h * P:(h + 1) * P],
                                rhs=S_sb[:, mi, :],
                                start=False, stop=(last and h == 1),
                                skip_group_check=True)
                    sum_in = mpool.tile([P, 2, P], BF16, tag="sum_in")
                    nc.vector.tensor_tensor(out=sum_in[:], in0=pside[:, 0:2, :],
                                            in1=eg[:], op=AL.add)
                    bi_in = mpool.tile([P, 2, P], BF16, tag="bi_in")
                    nc.vector.tensor_tensor(out=bi_in[:], in0=pside[:, 0:2, :],
                                            in1=eg[:], op=AL.mult)
                    pz = pz_pool.tile([P, 4, P], F32, tag="Z")
                    for i in range(4):
                        nc.tensor.matmul(out=pz[:, i, :],
                                         lhsT=bvec_sb[:, i * P:(i + 1) * P],
                                         rhs=ones_sb[:, :],
                                         start=(i == 0), stop=False,
                                         skip_group_check=True)
                    for br, xin in ((0, sum_in), (1, bi_in)):
                        for k in range(2):
                            for jh in range(2):
                                nc.tensor.matmul(
                                    out=pz[:, br * 2 + jh, :],
                                    lhsT=w12_sb[:, k,
                                                br * D + jh * P:
                                                br * D + (jh + 1) * P],
                                    rhs=xin[:, k, :],
                                    start=False, stop=(k == 1),
                                    skip_group_check=True)
                    ob = mpool.tile([P, 2, P], BF16, tag="ob")
                    if LRELU_MODE == 'act_lrelu':
                        ot = mpool.tile([P, 4, P], BF16, tag="ot")
                        nc.scalar.activation(out=ot[:, 0:2, :], in_=pz[:, 0:2, :],
                                             func=AF.Lrelu, alpha=NEG_SLOPE)
                        nc.scalar.activation(out=ot[:, 2:4, :], in_=pz[:, 2:4, :],
                                             func=AF.Lrelu, alpha=NEG_SLOPE)
                        nc.vector.tensor_tensor(out=ob[:], in0=ot[:, 0:2, :],
                                                in1=ot[:, 2:4, :], op=AL.add)
                    else:
                        # LR(z) = Relu(z) - slope*Relu(-z), per branch
                        rp = mpool.tile([P, 4, P], BF16, tag="rp")
                        rn = mpool.tile([P, 4, P], F32, tag="rn")
                        nc.scalar.activation(out=rp[:, 0:2, :], in_=pz[:, 0:2, :],
                                             func=AF.Relu)
                        nc.scalar.activation(out=rp[:, 2:4, :], in_=pz[:, 2:4, :],
                                             func=AF.Relu)
                        nc.scalar.activation(out=rn[:, 0:2, :], in_=pz[:, 0:2, :],
                                             func=AF.Relu, scale=-1.0)
                        nc.scalar.activation(out=rn[:, 2:4, :], in_=pz[:, 2:4, :],
                                             func=AF.Relu, scale=-1.0)
                        nc.vector.tensor_tensor(out=ob[:], in0=rp[:, 0:2, :],
                                                in1=rp[:, 2:4, :], op=AL.add)
                        rs = mpool.tile([P, 2, P], F32, tag="rs")
                        nc.vector.tensor_tensor(out=rs[:], in0=rn[:, 0:2, :],
                                                in1=rn[:, 2:4, :], op=AL.add)
                        nc.vector.tensor_scalar(out=rs[:], in0=rs[:],
                                                scalar1=-NEG_SLOPE, scalar2=None,
                                                op0=AL.mult)
                        nc.vector.tensor_tensor(out=ob[:], in0=ob[:],
                                                in1=rs[:], op=AL.add)
                    nc.sync.dma_start(out=outT[b, :, :, :], in_=ob[:])
    nc.compile()
    return nc


# ---------------- PJRT execution ----------------

def _make_exec(nc, device):
    import jax
    from concourse.bass2jax import _bass_exec_p, install_neuronx_cc_hook
    install_neuronx_cc_hook()
    in_names, out_names, out_avals, zero_outs = [], [], [], []
    in_specs = {}
    for alloc in nc.m.functions[0].allocations:
        if not isinstance(alloc, mybir.MemoryLocationSet):
            continue
        name = alloc.memorylocations[0].name
        if alloc.kind == "ExternalInput":
            in_names.append(name)
            in_specs[name] = (tuple(alloc.tensor_shape), mybir.dt.np(alloc.dtype))
        elif alloc.kind == "ExternalOutput":
            out_names.append(name)
            shape = tuple(alloc.tensor_shape)
            dtype = mybir.dt.np(alloc.dtype)
            out_avals.append(jax.core.ShapedArray(shape, dtype))
            zero_outs.append(np.zeros(shape, dtype))
    all_in_names = in_names + out_names

    def _body(*args):
        outs = _bass_exec_p.bind(
            *args,
            out_avals=tuple(out_avals),
            in_names=tuple(all_in_names),
            out_names=tuple(out_names),
            lowering_input_output_aliases=(),
            sim_require_finite=True,
            sim_require_nnan=True,
            nc=nc,
        )
        return tuple(outs)

    jitted = jax.jit(_body, keep_unused=True, device=device)
    return jitted, in_names, out_names, zero_outs, in_specs


class CoreRunner:
    def __init__(self, nc, device, in_map):
        import jax
        self.jax = jax
        (self.jitted, self.in_names, self.out_names, self.zero_outs,
         in_specs) = _make_exec(nc, device)
        self.dev_in = [
            jax.device_put(
                np.asarray(in_map[n]) if n in in_map
                else np.zeros(*in_specs[n][:1], in_specs[n][1]), device)
            for n in self.in_names]
        self.dev_zero = [jax.device_put(z, device) for z in self.zero_outs]

    def run_async(self):
        return self.jitted(*self.dev_in, *self.dev_zero)

    def outputs_np(self):
        outs = self.jax.block_until_ready(self.run_async())
        return {n: np.asarray(o) for n, o in zip(self.out_names, outs)}


# ---------------- top-level entry ----------------

def kernel(ego_embeddings, edge_vals, W1, b1, W2, b2, edge_rows, edge_cols):
    import jax
    ego = np.asarray(ego_embeddings, np.float32)
    edge_vals = np.asarray(edge_vals, np.float32)
    W1 = np.asarray(W1, np.float32); b1 = np.asarray(b1, np.float32)
    W2 = np.asarray(W2, np.float32); b2 = np.asarray(b2, np.float32)
    rows = np.asarray(edge_rows); cols = np.asarray(edge_cols)
    n = ego.shape[0]
    table_bf16 = make_table(ego)

    bounds = [round(n * c / N_CORES) for c in range(N_CORES + 1)]
    structs = [preprocess_core(rows, cols, edge_vals, bounds[c], bounds[c + 1])
               for c in range(N_CORES)]
    devices = jax.devices()[:N_CORES]

    ncs = [None] * N_CORES
    errs = [None] * N_CORES

    def _build(c):
        try:
            ncs[c] = build_core_program(structs[c], n)
        except Exception as e:  # noqa: BLE001
            errs[c] = e

    threads = [threading.Thread(target=_build, args=(c,)) for c in range(N_CORES)]
    for t in threads:
        t.start()
    for t in threads:
        t.join()
    for e in errs:
        if e is not None:
            raise e

    runners = []
    for c in range(N_CORES):
        in_map = make_core_inputs(structs[c], table_bf16,
                                  ego[bounds[c]:bounds[c + 1]], W1, b1, W2, b2)
        runners.append(CoreRunner(ncs[c], devices[c], in_map))

    global _LAST_RUNNERS, _LAST_NCS
    _LAST_RUNNERS = runners
    _LAST_NCS = ncs
    futs = [r.run_async() for r in runners]
    out = np.empty((n, D), np.float32)
    for c, (r, f) in enumerate(zip(runners, futs)):
        outs = jax.block_until_ready(f)
        oT = np.asarray(outs[r.out_names.index("outT")])  # [nb,128,2,128] bf16
        nnodes = bounds[c + 1] - bounds[c]
        nbb = oT.shape[0]
        full = oT.astype(np.float32).transpose(0, 3, 2, 1).reshape(nbb * P, D)
        out[bounds[c]:bounds[c + 1]] = full[:nnodes]
    return out
